# revision 1
# baseline (speedup 1.0000x reference)
"""Single-head causal attention on 8 TRN2 NeuronCores.

Problem: nn_AttentionHead (B=8, S=2048, D_MODEL=2048, HEAD_DIM=128), f32.
Sharding: data-parallel over batch -- one batch element per core, no
collectives.

Per-core algorithm (batch element b = core id):
  x        = hidden_state[b]            [S, D]   (bf16 on device)
  xT tiles = DMA-transpose loads        [D, S]   16 x [128, 2048] in SBUF
  qT = (Wq/sqrt(H)).T @ x.T + bq'       [H, S]   (scale folded into Wq)
  kT = Wk.T @ x.T + bk                  [H, S]
  v  = x @ Wv + bv                      [S, H]   (bias via K=1 matmul)
  scoresT_j = kT_j.T @ qT               [sk=128, sq>=j*128]  causal blocks only
  expT_j = exp(scoresT_j + diag mask)   bf16, feeds AV matmul as lhsT
  out_i = sum_j expT_j(block i).T @ [v_j | 1]   -> [sq=128, H+1]
  out   = out_i[:, :H] / out_i[:, H]    (ones column = softmax denominator)

All matmuls bf16 (PSUM accumulates f32).  No max-subtraction in softmax:
scores ~ N(0,1) so exp() cannot overflow f32.
"""

import sys

for _p in ("/opt/trn_rl_repo", "/opt/trn_rl_repo/concourse"):
    if _p not in sys.path:
        sys.path.insert(0, _p)

import ml_dtypes
import numpy as np

B, S, D, H = 8, 2048, 2048, 128
P = 128                 # partition size
DC = D // P             # d-chunks (16)
NT = S // P             # s-tiles (16)
NEG = -1.0e9
N_CORES = 8

BF16 = ml_dtypes.bfloat16


def build_graph():
    import concourse.bass as bass
    import concourse.mybir as mybir
    import concourse.tile as tile
    from concourse import bacc

    f32 = mybir.dt.float32
    bf16 = mybir.dt.bfloat16

    nc = bacc.Bacc("TRN2", target_bir_lowering=False, debug=False)

    x_ext = nc.declare_dram_parameter("x", [S, D], bf16, isOutput=False)
    wq_ext = nc.declare_dram_parameter("wq", [D, H], bf16, isOutput=False)
    wk_ext = nc.declare_dram_parameter("wk", [D, H], bf16, isOutput=False)
    wv_ext = nc.declare_dram_parameter("wv", [D, H], bf16, isOutput=False)
    bq_ext = nc.declare_dram_parameter("bq", [H], f32, isOutput=False)
    bk_ext = nc.declare_dram_parameter("bk", [H], f32, isOutput=False)
    bv_ext = nc.declare_dram_parameter("bv", [H], bf16, isOutput=False)
    mask_ext = nc.declare_dram_parameter("mask", [P, P], f32, isOutput=False)
    out_ext = nc.declare_dram_parameter("out", [S, H], f32, isOutput=True)

    with tile.TileContext(nc) as tc:
        with (
            tc.tile_pool(name="xt", bufs=1) as xt_pool,
            tc.tile_pool(name="wts", bufs=1) as w_pool,
            tc.tile_pool(name="qk", bufs=1) as qk_pool,
            tc.tile_pool(name="vp", bufs=1) as v_pool,
            tc.tile_pool(name="et", bufs=1) as e_pool,
            tc.tile_pool(name="ob", bufs=1) as o_pool,
            tc.tile_pool(name="sm", bufs=2) as small_pool,
            tc.tile_pool(name="psb", bufs=3, space="PSUM") as pp_big,
            tc.tile_pool(name="psv", bufs=2, space="PSUM") as pp_v,
            tc.tile_pool(name="pso", bufs=2, space="PSUM") as pp_o,
        ):
            # ---- constant / weight loads -------------------------------
            wq_sb = w_pool.tile([P, DC, H], bf16, tag="wq")
            wk_sb = w_pool.tile([P, DC, H], bf16, tag="wk")
            wv_sb = w_pool.tile([P, DC, H], bf16, tag="wv")
            nc.sync.dma_start(wq_sb[:], wq_ext.rearrange("(c p) h -> p c h", p=P))
            nc.sync.dma_start(wk_sb[:], wk_ext.rearrange("(c p) h -> p c h", p=P))
            nc.sync.dma_start(wv_sb[:], wv_ext.rearrange("(c p) h -> p c h", p=P))

            bq_sb = w_pool.tile([P, 1], f32, tag="bq")
            bk_sb = w_pool.tile([P, 1], f32, tag="bk")
            nc.sync.dma_start(bq_sb[:], bq_ext.rearrange("(p o) -> p o", o=1))
            nc.sync.dma_start(bk_sb[:], bk_ext.rearrange("(p o) -> p o", o=1))
            bv_sb = w_pool.tile([1, H], bf16, tag="bv")
            nc.sync.dma_start(bv_sb[:], bv_ext.rearrange("(o h) -> o h", o=1))
            mask_sb = w_pool.tile([P, P], f32, tag="mask")
            nc.sync.dma_start(mask_sb[:], mask_ext[:])
            ones1_sb = w_pool.tile([1, H], bf16, tag="ones1")
            nc.vector.memset(ones1_sb[:], 1.0)

            # ---- x.T tiles via DMA transpose ---------------------------
            xt = []
            for c in range(DC):
                t = xt_pool.tile([P, S], bf16, tag=f"xt{c}")
                nc.sync.dma_start(
                    t[:], x_ext[:, c * P : (c + 1) * P], transpose=True
                )
                xt.append(t)

            # ---- kT / qT projections -----------------------------------
            kT_sb = qk_pool.tile([P, S], bf16, tag="kT")
            qT_sb = qk_pool.tile([P, S], bf16, tag="qT")
            for name, w_sb, b_sb, dst in (
                ("k", wk_sb, bk_sb, kT_sb),
                ("q", wq_sb, bq_sb, qT_sb),
            ):
                for n in range(S // 512):
                    ps = pp_big.tile([P, 512], f32, tag="big")
                    for c in range(DC):
                        nc.tensor.matmul(
                            ps[:],
                            wq_sb[:, c, :] if name == "q" else wk_sb[:, c, :],
                            xt[c][:, n * 512 : (n + 1) * 512],
                            start=(c == 0),
                            stop=(c == DC - 1),
                        )
                    nc.vector.tensor_scalar_add(
                        dst[:, n * 512 : (n + 1) * 512], ps[:], b_sb[:]
                    )

            # ---- v projection (natural layout, bias via K=1 matmul) ----
            v_sb = v_pool.tile([P, NT, H + 1], bf16, tag="v")
            nc.vector.memset(v_sb[:, :, H], 1.0)
            for j in range(NT):
                ps = pp_v.tile([P, H], f32, tag="vps")
                nc.tensor.matmul(
                    ps[:], ones1_sb[0:1, :], bv_sb[0:1, :], start=True, stop=False
                )
                for c in range(DC):
                    nc.tensor.matmul(
                        ps[:],
                        xt[c][:, j * P : (j + 1) * P],
                        wv_sb[:, c, :],
                        start=False,
                        stop=(c == DC - 1),
                    )
                nc.vector.tensor_copy(v_sb[:, j, 0:H], ps[:])

            # ---- causal scoresT + exp ----------------------------------
            expT = []
            for j in range(NT):
                width = (NT - j) * P
                et = e_pool.tile([P, width], bf16, tag=f"expT{j}")
                expT.append(et)
                off = 0
                while off < width:
                    w = min(512, width - off)
                    ps = pp_big.tile([P, 512], f32, tag="big")
                    nc.tensor.matmul(
                        ps[:, 0:w],
                        kT_sb[:, j * P : (j + 1) * P],
                        qT_sb[:, j * P + off : j * P + off + w],
                        start=True,
                        stop=True,
                    )
                    if off == 0:
                        nc.vector.tensor_add(ps[:, 0:P], ps[:, 0:P], mask_sb[:])
                    nc.scalar.activation(
                        et[:, off : off + w],
                        ps[:, 0:w],
                        mybir.ActivationFunctionType.Exp,
                    )
                    off += w

            # ---- attention @ v, softmax divide -------------------------
            out_sb = o_pool.tile([P, NT, H], f32, tag="out")
            for i in range(NT):
                ps = pp_o.tile([P, H + 1], f32, tag="ops")
                for j in range(i + 1):
                    nc.tensor.matmul(
                        ps[:],
                        expT[j][:, (i - j) * P : (i - j + 1) * P],
                        v_sb[:, j, :],
                        start=(j == 0),
                        stop=(j == i),
                    )
                recip = small_pool.tile([P, 1], f32, tag="recip")
                nc.vector.reciprocal(recip[:], ps[:, H : H + 1])
                nc.vector.tensor_scalar_mul(out_sb[:, i, :], ps[:, 0:H], recip[:])

            nc.sync.dma_start(
                out_ext.rearrange("(i p) h -> p i h", p=P), out_sb[:]
            )

    nc.compile()
    return nc


_cached = {}


def _get_graph():
    if "nc" not in _cached:
        _cached["nc"] = build_graph()
    return _cached["nc"]


def kernel(hidden_state, Wq, bq, Wk, bk, Wv, bv):
    from concourse.bass_utils import run_bass_kernel_spmd

    hs = np.asarray(hidden_state, dtype=np.float32)
    scale = np.float32(1.0 / np.sqrt(np.float32(H)))
    wq = (np.asarray(Wq, dtype=np.float32) * scale).astype(BF16)
    wk = np.asarray(Wk, dtype=np.float32).astype(BF16)
    wv = np.asarray(Wv, dtype=np.float32).astype(BF16)
    bq_s = (np.asarray(bq, dtype=np.float32) * scale).astype(np.float32)
    bk_f = np.asarray(bk, dtype=np.float32)
    bv_b = np.asarray(bv, dtype=np.float32).astype(BF16)
    r = np.arange(P)
    mask = np.where(r[:, None] > r[None, :], np.float32(NEG), np.float32(0.0))
    mask = mask.astype(np.float32)

    in_maps = []
    for b in range(N_CORES):
        in_maps.append(
            {
                "x": hs[b].astype(BF16),
                "wq": wq,
                "wk": wk,
                "wv": wv,
                "bq": bq_s,
                "bk": bk_f,
                "bv": bv_b,
                "mask": mask,
            }
        )

    nc = _get_graph()
    res = run_bass_kernel_spmd(nc, in_maps, core_ids=list(range(N_CORES)))
    out = np.stack([res.results[i]["out"] for i in range(N_CORES)], axis=0)
    return out.astype(np.float32)


def run_traced(hidden_state, Wq, bq, Wk, bk, Wv, bv):
    """Like kernel() but with NTFF tracing; returns (out, BassKernelResults)."""
    from concourse.bass_utils import run_bass_kernel_spmd

    hs = np.asarray(hidden_state, dtype=np.float32)
    scale = np.float32(1.0 / np.sqrt(np.float32(H)))
    wq = (np.asarray(Wq, dtype=np.float32) * scale).astype(BF16)
    wk = np.asarray(Wk, dtype=np.float32).astype(BF16)
    wv = np.asarray(Wv, dtype=np.float32).astype(BF16)
    bq_s = (np.asarray(bq, dtype=np.float32) * scale).astype(np.float32)
    bk_f = np.asarray(bk, dtype=np.float32)
    bv_b = np.asarray(bv, dtype=np.float32).astype(BF16)
    r = np.arange(P)
    mask = np.where(r[:, None] > r[None, :], np.float32(NEG), np.float32(0.0)).astype(
        np.float32
    )
    in_maps = [
        {
            "x": hs[b].astype(BF16),
            "wq": wq,
            "wk": wk,
            "wv": wv,
            "bq": bq_s,
            "bk": bk_f,
            "bv": bv_b,
            "mask": mask,
        }
        for b in range(N_CORES)
    ]
    nc = _get_graph()
    res = run_bass_kernel_spmd(nc, in_maps, core_ids=list(range(N_CORES)), trace=True)
    out = np.stack([res.results[i]["out"] for i in range(N_CORES)], axis=0).astype(
        np.float32
    )
    return out, res


# revision 4
# speedup vs baseline: 1.0834x; 1.0834x over previous
"""Single-head causal attention on 8 TRN2 NeuronCores.

Problem: nn_AttentionHead (B=8, S=2048, D_MODEL=2048, HEAD_DIM=128), f32.
Sharding: data-parallel over batch -- one batch element per core, no
collectives.

Per-core algorithm (batch element b = core id):
  x        = hidden_state[b]            [S, D]   (bf16, d-chunk-major layout)
  xT tiles = DMA-transpose loads        [D, S]   16 x [128, 2048] in SBUF
  qT = (Wq/sqrt(H)).T @ x.T + bq'       [H, S]   (scale folded into Wq)
  kT = Wk.T @ x.T + bk                  [H, S]
  v  = x @ Wv + bv                      [S, H]   (bias via K=1 matmul)
  scoresT_j = kT_j.T @ qT               [sk=128, sq>=j*128]  causal blocks only
  expT_j = exp(scoresT_j + diag mask)   bf16, feeds AV matmul as lhsT
  out_i = sum_j expT_j(block i).T @ [v_j | 1]   -> [sq=128, H+1]
  out   = out_i[:, :H] / out_i[:, H]    (ones column = softmax denominator)

All matmuls bf16 (PSUM accumulates f32).  No max-subtraction in softmax:
scores ~ N(0,1) so exp() cannot overflow f32.  Program order keeps the
TensorEngine dense (HAM stays warm) and pipelines exp (ACT) under PE.
"""

import sys

for _p in ("/opt/trn_rl_repo", "/opt/trn_rl_repo/concourse"):
    if _p not in sys.path:
        sys.path.insert(0, _p)

import ml_dtypes
import numpy as np

B, S, D, H = 8, 2048, 2048, 128
P = 128                 # partition size
DC = D // P             # d-chunks (16)
NT = S // P             # s-tiles (16)
NEG = -1.0e9
N_CORES = 8

BF16 = ml_dtypes.bfloat16


def build_graph():
    import concourse.bass as bass
    import concourse.mybir as mybir
    import concourse.tile as tile
    from concourse import bacc

    f32 = mybir.dt.float32
    bf16 = mybir.dt.bfloat16
    Exp = mybir.ActivationFunctionType.Exp

    nc = bacc.Bacc("TRN2", target_bir_lowering=False, debug=False)

    # x in d-chunk-major layout: x_ext[c, s, p] = x[s, c*128+p] so each
    # transpose-DMA reads a fully contiguous 512 KB block.
    x_ext = nc.declare_dram_parameter("x", [DC, S, P], bf16, isOutput=False)
    # weights pre-arranged host-side to [P, DC*H]: w_ext[p, c*H+h] = W[c*128+p, h]
    wq_ext = nc.declare_dram_parameter("wq", [P, DC * H], bf16, isOutput=False)
    wk_ext = nc.declare_dram_parameter("wk", [P, DC * H], bf16, isOutput=False)
    wv_ext = nc.declare_dram_parameter("wv", [P, DC * H], bf16, isOutput=False)
    bq_ext = nc.declare_dram_parameter("bq", [H], f32, isOutput=False)
    bk_ext = nc.declare_dram_parameter("bk", [H], f32, isOutput=False)
    bv_ext = nc.declare_dram_parameter("bv", [H], bf16, isOutput=False)
    mask_ext = nc.declare_dram_parameter("mask", [P, P], f32, isOutput=False)
    out_ext = nc.declare_dram_parameter("out", [S, H], f32, isOutput=True)

    with tile.TileContext(nc) as tc:
        with (
            tc.tile_pool(name="xt", bufs=1) as xt_pool,
            tc.tile_pool(name="wts", bufs=1) as w_pool,
            tc.tile_pool(name="qk", bufs=1) as qk_pool,
            tc.tile_pool(name="vp", bufs=1) as v_pool,
            tc.tile_pool(name="et", bufs=1) as e_pool,
            tc.tile_pool(name="ob", bufs=1) as o_pool,
            tc.tile_pool(name="sm", bufs=4) as small_pool,
            tc.tile_pool(name="psb", bufs=3, space="PSUM") as pp_big,
            tc.tile_pool(name="psv", bufs=2, space="PSUM") as pp_v,
            tc.tile_pool(name="pso", bufs=2, space="PSUM") as pp_o,
        ):
            # ---- constant / weight loads (scalar HWDGE queue) ----------
            wq_sb = w_pool.tile([P, DC, H], bf16, tag="wq")
            wk_sb = w_pool.tile([P, DC, H], bf16, tag="wk")
            wv_sb = w_pool.tile([P, DC, H], bf16, tag="wv")
            bq_sb = w_pool.tile([P, 1], f32, tag="bq")
            bk_sb = w_pool.tile([P, 1], f32, tag="bk")
            bv_sb = w_pool.tile([1, H], bf16, tag="bv")
            mask_sb = w_pool.tile([P, P], f32, tag="mask")
            ones1_sb = w_pool.tile([1, H], bf16, tag="ones1")
            nc.sync.dma_start(wq_sb[:], wq_ext.rearrange("p (c h) -> p c h", h=H))
            nc.sync.dma_start(bq_sb[:], bq_ext.rearrange("(p o) -> p o", o=1))
            nc.sync.dma_start(wk_sb[:], wk_ext.rearrange("p (c h) -> p c h", h=H))
            nc.sync.dma_start(bk_sb[:], bk_ext.rearrange("(p o) -> p o", o=1))
            nc.sync.dma_start(wv_sb[:], wv_ext.rearrange("p (c h) -> p c h", h=H))
            nc.sync.dma_start(bv_sb[:], bv_ext.rearrange("(o h) -> o h", o=1))
            nc.sync.dma_start(mask_sb[:], mask_ext[:])
            nc.vector.memset(ones1_sb[:], 1.0)

            # ---- x.T tiles via DMA transpose, split over both HWDGE ----
            xt = []
            for c in range(DC):
                t = xt_pool.tile([P, S], bf16, tag=f"xt{c}")
                nc.sync.dma_start(t[:], x_ext[c], transpose=True)
                xt.append(t)

            # ---- qT projection (needed in full by every scores tile) ---
            kT_sb = qk_pool.tile([P, S], bf16, tag="kT")
            qT_sb = qk_pool.tile([P, S], bf16, tag="qT")
            for n in range(S // 512):
                ps = pp_big.tile([P, 512], f32, tag="big")
                for c in range(DC):
                    nc.tensor.matmul(
                        ps[:],
                        wq_sb[:, c, :],
                        xt[c][:, n * 512 : (n + 1) * 512],
                        start=(c == 0),
                        stop=(c == DC - 1),
                    )
                nc.vector.tensor_scalar_add(
                    qT_sb[:, n * 512 : (n + 1) * 512], ps[:], bq_sb[:]
                )

            # ---- kT projection ----------------------------------------
            for n in range(S // 512):
                ps = pp_big.tile([P, 512], f32, tag="big")
                for c in range(DC):
                    nc.tensor.matmul(
                        ps[:],
                        wk_sb[:, c, :],
                        xt[c][:, n * 512 : (n + 1) * 512],
                        start=(c == 0),
                        stop=(c == DC - 1),
                    )
                nc.vector.tensor_scalar_add(
                    kT_sb[:, n * 512 : (n + 1) * 512], ps[:], bk_sb[:]
                )

            # ---- v projection (bias via K=1 matmul with ones row) ------
            v_sb = v_pool.tile([P, NT, H + 1], bf16, tag="v")
            nc.vector.memset(v_sb[:, :, H], 1.0)
            out_sb = o_pool.tile([P, NT, H], f32, tag="out")
            expT = [None] * NT

            for j in range(NT):
                ps_v = pp_v.tile([P, H], f32, tag="vps")
                nc.tensor.matmul(
                    ps_v[:], ones1_sb[0:1, :], bv_sb[0:1, :],
                    start=True, stop=False,
                )
                for c in range(DC):
                    nc.tensor.matmul(
                        ps_v[:],
                        xt[c][:, j * P : (j + 1) * P],
                        wv_sb[:, c, :],
                        start=False,
                        stop=(c == DC - 1),
                    )
                nc.vector.tensor_copy(v_sb[:, j, 0:H], ps_v[:])

            # ---- causal scoresT + exp ----------------------------------
            for j in range(NT):
                width = (NT - j) * P
                et = e_pool.tile([P, width], bf16, tag=f"expT{j}")
                expT[j] = et
                off = 0
                while off < width:
                    w = min(512, width - off)
                    ps_s = pp_big.tile([P, 512], f32, tag="big")
                    nc.tensor.matmul(
                        ps_s[:, 0:w],
                        kT_sb[:, j * P : (j + 1) * P],
                        qT_sb[:, j * P + off : j * P + off + w],
                        start=True,
                        stop=True,
                    )
                    if off == 0:
                        nc.vector.tensor_add(
                            ps_s[:, 0:P], ps_s[:, 0:P], mask_sb[:]
                        )
                    nc.scalar.activation(et[:, off : off + w], ps_s[:, 0:w], Exp)
                    off += w

            # ---- attention @ v, softmax divide -------------------------
            for i in range(NT):
                ps_o = pp_o.tile([P, H + 1], f32, tag="ops")
                for jj in range(i + 1):
                    nc.tensor.matmul(
                        ps_o[:],
                        expT[jj][:, (i - jj) * P : (i - jj + 1) * P],
                        v_sb[:, jj, :],
                        start=(jj == 0),
                        stop=(jj == i),
                    )
                recip = small_pool.tile([P, 1], f32, tag="recip")
                nc.vector.reciprocal(recip[:], ps_o[:, H : H + 1])
                nc.vector.tensor_scalar_mul(
                    out_sb[:, i, :], ps_o[:, 0:H], recip[:]
                )

            nc.sync.dma_start(
                out_ext.rearrange("(i p) h -> p i h", p=P), out_sb[:]
            )

    nc.compile()
    return nc


_cached = {}


def _get_graph():
    if "nc" not in _cached:
        _cached["nc"] = build_graph()
    return _cached["nc"]


def _prep_inputs(hidden_state, Wq, bq, Wk, bk, Wv, bv):
    hs = np.asarray(hidden_state, dtype=np.float32)
    scale = np.float32(1.0 / np.sqrt(np.float32(H)))

    def prep_w(w, s=None):
        w = np.asarray(w, dtype=np.float32)
        if s is not None:
            w = w * s
        # [D, H] -> [P, DC*H] with w_out[p, c*H+h] = w[c*P+p, h]
        return np.ascontiguousarray(
            w.reshape(DC, P, H).transpose(1, 0, 2).reshape(P, DC * H)
        ).astype(BF16)

    wq = prep_w(Wq, scale)
    wk = prep_w(Wk)
    wv = prep_w(Wv)
    bq_s = (np.asarray(bq, dtype=np.float32) * scale).astype(np.float32)
    bk_f = np.asarray(bk, dtype=np.float32)
    bv_b = np.asarray(bv, dtype=np.float32).astype(BF16)
    r = np.arange(P)
    mask = np.where(r[:, None] > r[None, :], np.float32(NEG), np.float32(0.0)).astype(
        np.float32
    )

    in_maps = []
    for b in range(N_CORES):
        # x -> d-chunk-major [DC, S, P] so transpose DMAs read contiguously
        xb = np.ascontiguousarray(
            hs[b].astype(BF16).reshape(S, DC, P).transpose(1, 0, 2)
        )
        in_maps.append(
            {
                "x": xb,
                "wq": wq,
                "wk": wk,
                "wv": wv,
                "bq": bq_s,
                "bk": bk_f,
                "bv": bv_b,
                "mask": mask,
            }
        )
    return in_maps


def kernel(hidden_state, Wq, bq, Wk, bk, Wv, bv):
    from concourse.bass_utils import run_bass_kernel_spmd

    in_maps = _prep_inputs(hidden_state, Wq, bq, Wk, bk, Wv, bv)
    nc = _get_graph()
    res = run_bass_kernel_spmd(nc, in_maps, core_ids=list(range(N_CORES)))
    out = np.stack([res.results[i]["out"] for i in range(N_CORES)], axis=0)
    return out.astype(np.float32)


def run_traced(hidden_state, Wq, bq, Wk, bk, Wv, bv):
    """Like kernel() but with NTFF tracing; returns (out, BassKernelResults)."""
    from concourse.bass_utils import run_bass_kernel_spmd

    in_maps = _prep_inputs(hidden_state, Wq, bq, Wk, bk, Wv, bv)
    nc = _get_graph()
    res = run_bass_kernel_spmd(nc, in_maps, core_ids=list(range(N_CORES)), trace=True)
    out = np.stack([res.results[i]["out"] for i in range(N_CORES)], axis=0).astype(
        np.float32
    )
    return out, res


# revision 7
# speedup vs baseline: 1.3412x; 1.2380x over previous
"""Single-head causal attention on 8 TRN2 NeuronCores.

Problem: nn_AttentionHead (B=8, S=2048, D_MODEL=2048, HEAD_DIM=128), f32.
Sharding: data-parallel over batch -- one batch element per core, no
collectives.

Per-core algorithm (batch element b = core id):
  x        = hidden_state[b]            [S, D]   (bf16, d-chunk-major layout)
  xT tiles = DMA-transpose loads        [D, S]   16 x [128, 2048] in SBUF
  qT = (Wq/sqrt(H)).T @ x.T + bq'       [H, S]   (scale folded into Wq)
  kT = Wk.T @ x.T + bk                  [H, S]
  v  = x @ Wv + bv                      [S, H]   (bias via K=1 matmul)
  scoresT_j = kT_j.T @ qT               [sk=128, sq>=j*128]  causal blocks only
  expT_j = exp(scoresT_j + diag mask)   bf16, feeds AV matmul as lhsT
  out_i = sum_j expT_j(block i).T @ [v_j | 1]   -> [sq=128, H+1]
  out   = out_i[:, :H] / out_i[:, H]    (ones column = softmax denominator)

All matmuls bf16 (PSUM accumulates f32).  No max-subtraction in softmax:
scores ~ N(0,1) so exp() cannot overflow f32.  Program order keeps the
TensorEngine dense (HAM stays warm) and pipelines exp (ACT) under PE.
"""

import sys

for _p in ("/opt/trn_rl_repo", "/opt/trn_rl_repo/concourse"):
    if _p not in sys.path:
        sys.path.insert(0, _p)

import ml_dtypes
import numpy as np

B, S, D, H = 8, 2048, 2048, 128
P = 128                 # partition size
DC = D // P             # d-chunks (16)
NT = S // P             # s-tiles (16)
NEG = -1.0e9
N_CORES = 8

BF16 = ml_dtypes.bfloat16


def build_graph():
    import concourse.bass as bass
    import concourse.mybir as mybir
    import concourse.tile as tile
    from concourse import bacc

    f32 = mybir.dt.float32
    bf16 = mybir.dt.bfloat16
    Exp = mybir.ActivationFunctionType.Exp

    nc = bacc.Bacc("TRN2", target_bir_lowering=False, debug=False)

    # x in d-chunk-major layout: x_ext[c, s, p] = x[s, c*128+p] so each
    # transpose-DMA reads a fully contiguous 512 KB block.
    x_ext = nc.declare_dram_parameter("x", [DC, S, P], bf16, isOutput=False)
    # weights pre-arranged host-side to [P, DC*H]: w_ext[p, c*H+h] = W[c*128+p, h]
    wq_ext = nc.declare_dram_parameter("wq", [P, DC * H], bf16, isOutput=False)
    wk_ext = nc.declare_dram_parameter("wk", [P, DC * H], bf16, isOutput=False)
    wv_ext = nc.declare_dram_parameter("wv", [P, DC * H], bf16, isOutput=False)
    bq_ext = nc.declare_dram_parameter("bq", [H], f32, isOutput=False)
    bk_ext = nc.declare_dram_parameter("bk", [H], f32, isOutput=False)
    bv_ext = nc.declare_dram_parameter("bv", [H], bf16, isOutput=False)
    mask_ext = nc.declare_dram_parameter("mask", [P, P], f32, isOutput=False)
    out_ext = nc.declare_dram_parameter("out", [S, H], f32, isOutput=True)

    with tile.TileContext(nc) as tc:
        with (
            tc.tile_pool(name="xt", bufs=1) as xt_pool,
            tc.tile_pool(name="wts", bufs=1) as w_pool,
            tc.tile_pool(name="qk", bufs=1) as qk_pool,
            tc.tile_pool(name="vp", bufs=1) as v_pool,
            tc.tile_pool(name="et", bufs=1) as e_pool,
            tc.tile_pool(name="ob", bufs=1) as o_pool,
            tc.tile_pool(name="sm", bufs=4) as small_pool,
        ):
            # ---- constant / weight loads (scalar HWDGE queue) ----------
            wq_sb = w_pool.tile([P, DC, H], bf16, tag="wq")
            wk_sb = w_pool.tile([P, DC, H], bf16, tag="wk")
            wv_sb = w_pool.tile([P, DC, H], bf16, tag="wv")
            bq_sb = w_pool.tile([P, 1], f32, tag="bq")
            bk_sb = w_pool.tile([P, 1], f32, tag="bk")
            bv_sb = w_pool.tile([1, H], bf16, tag="bv")
            mask_sb = w_pool.tile([P, P], f32, tag="mask")
            ones1_sb = w_pool.tile([1, H], bf16, tag="ones1")
            nc.sync.dma_start(wq_sb[:], wq_ext.rearrange("p (c h) -> p c h", h=H))
            nc.sync.dma_start(bq_sb[:], bq_ext.rearrange("(p o) -> p o", o=1))
            nc.sync.dma_start(wk_sb[:], wk_ext.rearrange("p (c h) -> p c h", h=H))
            nc.sync.dma_start(bk_sb[:], bk_ext.rearrange("(p o) -> p o", o=1))
            nc.sync.dma_start(wv_sb[:], wv_ext.rearrange("p (c h) -> p c h", h=H))
            nc.sync.dma_start(bv_sb[:], bv_ext.rearrange("(o h) -> o h", o=1))
            nc.sync.dma_start(mask_sb[:], mask_ext[:])
            nc.vector.memset(ones1_sb[:], 1.0)

            # ---- x.T tiles via DMA transpose, split over both HWDGE ----
            xt = []
            for c in range(DC):
                t = xt_pool.tile([P, S], bf16, tag=f"xt{c}")
                nc.sync.dma_start(t[:], x_ext[c], transpose=True)
                xt.append(t)

            # ---- q+k projections, c-streaming under the transposes -----
            # 8 PSUM banks hold all q/k chunk accumulators; each arriving
            # xt[c] immediately feeds 8 matmuls, so the PE streams behind
            # the (serial) xbar transpose wall instead of waiting for it.
            kT_sb = qk_pool.tile([P, S], bf16, tag="kT")
            qT_sb = qk_pool.tile([P, S], bf16, tag="qT")
            with tc.tile_pool(name="pqk", bufs=1, space="PSUM") as pp_qk:
                qkps = [
                    pp_qk.tile(
                        [P, 512], f32, tag=f"qkps{i}", name=f"qkps{i}"
                    )
                    for i in range(8)
                ]
                for c in range(DC):
                    for n in range(4):
                        nc.tensor.matmul(
                            qkps[n][:],
                            wq_sb[:, c, :],
                            xt[c][:, n * 512 : (n + 1) * 512],
                            start=(c == 0),
                            stop=(c == DC - 1),
                        )
                        nc.tensor.matmul(
                            qkps[4 + n][:],
                            wk_sb[:, c, :],
                            xt[c][:, n * 512 : (n + 1) * 512],
                            start=(c == 0),
                            stop=(c == DC - 1),
                        )
                # kT chunk 0 first: scores_0 needs it plus all of qT
                nc.vector.tensor_scalar_add(kT_sb[:, 0:512], qkps[4][:], bk_sb[:])
                for n in range(4):
                    nc.vector.tensor_scalar_add(
                        qT_sb[:, n * 512 : (n + 1) * 512], qkps[n][:], bq_sb[:]
                    )
                for n in range(1, 4):
                    nc.vector.tensor_scalar_add(
                        kT_sb[:, n * 512 : (n + 1) * 512], qkps[4 + n][:], bk_sb[:]
                    )

            # ---- streaming phase 2: per j {v_j, scoresT_j, exp, AV} ----
            v_sb = v_pool.tile([P, NT, H + 1], bf16, tag="v")
            nc.vector.memset(v_sb[:, :, H], 1.0)
            out_sb = o_pool.tile([P, NT, H], f32, tag="out")
            expT = [None] * NT

            with (
                tc.tile_pool(name="pss", bufs=2, space="PSUM") as pp_s,
                tc.tile_pool(name="psv", bufs=2, space="PSUM") as pp_v,
                tc.tile_pool(name="pso", bufs=2, space="PSUM") as pp_o,
            ):
                for j in range(NT):
                    # v_j projection (bias via K=1 matmul with ones row)
                    ps_v = pp_v.tile([P, H], f32, tag="vps")
                    nc.tensor.matmul(
                        ps_v[:], ones1_sb[0:1, :], bv_sb[0:1, :],
                        start=True, stop=False,
                    )
                    for c in range(DC):
                        nc.tensor.matmul(
                            ps_v[:],
                            xt[c][:, j * P : (j + 1) * P],
                            wv_sb[:, c, :],
                            start=False,
                            stop=(c == DC - 1),
                        )
                    nc.vector.tensor_copy(v_sb[:, j, 0:H], ps_v[:])

                    # causal scoresT_j + exp
                    width = (NT - j) * P
                    et = e_pool.tile([P, width], bf16, tag=f"expT{j}")
                    expT[j] = et
                    off = 0
                    while off < width:
                        w = min(512, width - off)
                        ps_s = pp_s.tile([P, 512], f32, tag="sps")
                        nc.tensor.matmul(
                            ps_s[:, 0:w],
                            kT_sb[:, j * P : (j + 1) * P],
                            qT_sb[:, j * P + off : j * P + off + w],
                            start=True,
                            stop=True,
                        )
                        if off == 0:
                            nc.vector.tensor_add(
                                ps_s[:, 0:P], ps_s[:, 0:P], mask_sb[:]
                            )
                        nc.scalar.activation(
                            et[:, off : off + w], ps_s[:, 0:w], Exp
                        )
                        off += w

                    # AV row i=j (expT_0..j and v_0..j are all ready)
                    i = j
                    ps_o = pp_o.tile([P, H + 1], f32, tag="ops")
                    for jj in range(i + 1):
                        nc.tensor.matmul(
                            ps_o[:],
                            expT[jj][:, (i - jj) * P : (i - jj + 1) * P],
                            v_sb[:, jj, :],
                            start=(jj == 0),
                            stop=(jj == i),
                        )
                    recip = small_pool.tile([P, 1], f32, tag="recip")
                    nc.vector.reciprocal(recip[:], ps_o[:, H : H + 1])
                    nc.vector.tensor_scalar_mul(
                        out_sb[:, i, :], ps_o[:, 0:H], recip[:]
                    )

            nc.sync.dma_start(
                out_ext.rearrange("(i p) h -> p i h", p=P), out_sb[:]
            )

    nc.compile()
    return nc


_cached = {}


def _get_graph():
    if "nc" not in _cached:
        _cached["nc"] = build_graph()
    return _cached["nc"]


def _prep_inputs(hidden_state, Wq, bq, Wk, bk, Wv, bv):
    hs = np.asarray(hidden_state, dtype=np.float32)
    scale = np.float32(1.0 / np.sqrt(np.float32(H)))

    def prep_w(w, s=None):
        w = np.asarray(w, dtype=np.float32)
        if s is not None:
            w = w * s
        # [D, H] -> [P, DC*H] with w_out[p, c*H+h] = w[c*P+p, h]
        return np.ascontiguousarray(
            w.reshape(DC, P, H).transpose(1, 0, 2).reshape(P, DC * H)
        ).astype(BF16)

    wq = prep_w(Wq, scale)
    wk = prep_w(Wk)
    wv = prep_w(Wv)
    bq_s = (np.asarray(bq, dtype=np.float32) * scale).astype(np.float32)
    bk_f = np.asarray(bk, dtype=np.float32)
    bv_b = np.asarray(bv, dtype=np.float32).astype(BF16)
    r = np.arange(P)
    mask = np.where(r[:, None] > r[None, :], np.float32(NEG), np.float32(0.0)).astype(
        np.float32
    )

    in_maps = []
    for b in range(N_CORES):
        # x -> d-chunk-major [DC, S, P] so transpose DMAs read contiguously
        xb = np.ascontiguousarray(
            hs[b].astype(BF16).reshape(S, DC, P).transpose(1, 0, 2)
        )
        in_maps.append(
            {
                "x": xb,
                "wq": wq,
                "wk": wk,
                "wv": wv,
                "bq": bq_s,
                "bk": bk_f,
                "bv": bv_b,
                "mask": mask,
            }
        )
    return in_maps


def kernel(hidden_state, Wq, bq, Wk, bk, Wv, bv):
    from concourse.bass_utils import run_bass_kernel_spmd

    in_maps = _prep_inputs(hidden_state, Wq, bq, Wk, bk, Wv, bv)
    nc = _get_graph()
    res = run_bass_kernel_spmd(nc, in_maps, core_ids=list(range(N_CORES)))
    out = np.stack([res.results[i]["out"] for i in range(N_CORES)], axis=0)
    return out.astype(np.float32)


def run_traced(hidden_state, Wq, bq, Wk, bk, Wv, bv):
    """Like kernel() but with NTFF tracing; returns (out, BassKernelResults)."""
    from concourse.bass_utils import run_bass_kernel_spmd

    in_maps = _prep_inputs(hidden_state, Wq, bq, Wk, bk, Wv, bv)
    nc = _get_graph()
    res = run_bass_kernel_spmd(nc, in_maps, core_ids=list(range(N_CORES)), trace=True)
    out = np.stack([res.results[i]["out"] for i in range(N_CORES)], axis=0).astype(
        np.float32
    )
    return out, res
